# revision 4
# baseline (speedup 1.0000x reference)
"""GQA attention (B=2, S=2048, 16 q heads / 4 kv heads, head_dim=128) with RoPE
and causal softmax, tensor-parallel over heads x data-parallel over batch on
8 Trainium2 NeuronCores.

Core c (0..7): batch b = c//4, TP rank r = c%4.
Each core computes, for its batch and its 4 q heads / 1 kv head:
  QT/KT/VT projections (transposed layout, contraction on partitions),
  RoPE via a DVE partition pair-swap (stream_shuffle) + elementwise muls
  (sign of the swap folded into the host-prepared sin table),
  causal softmax without max-subtraction (scores bounded; exp in fp32->bf16),
  softmax denominator via DVE accumulation of exp tiles across key tiles
  + one ones-matmul per (head, q-block) for the partition reduction,
  P@V in transposed layout (no transposes of the probability tiles),
  causal diagonal handled at 128-column granularity,
  row-sharded output projection producing a partial [DIM, S] f32 output.
Host sums the 4 partials per batch (the row-parallel all-reduce) + transposes.
"""

import numpy as np
import ml_dtypes
from contextlib import ExitStack

import concourse.bass as bass
import concourse.tile as tile
from concourse import bacc, mybir, bass_utils, masks

B, S, DIM = 2, 2048, 2048
NH, NKV, HD = 16, 4, 128
TPR = 4            # tensor-parallel ranks per batch
LQH = NH // TPR    # 4 local q heads
QB = 512           # q block (free dim of matmuls)
NQB = S // QB      # 4
NDT = DIM // 128   # 16 contraction tiles for the projections
NKT = S // 128     # 16 key tiles
SCALE = 1.0 / float(np.sqrt(HD))
SWAP_MASK = [i ^ 1 for i in range(32)]  # partition pair swap within quadrants

BF = mybir.dt.bfloat16
F32 = mybir.dt.float32

# schedule/config constants (tuned via TimelineSim cost model)
F_ROPE_SHUF = True   # RoPE via DVE stream_shuffle (else psw matmul)
F_DIAG128 = True     # 128-col diagonal granularity (else 256 halves)
F_DEN_DVE = False    # denominator via full DVE accumulation (DVE-bound: off)
F_DEN_ENG = 'v'      # 'v' = vector engine, 'g' = gpsimd for den accumulation
F_DEN_QUAD = True    # denominator: bf16 group-accumulate fulls on DVE + 1 matmul/group
F_PSCR = 3
F_WONCE = True       # load weights/ce/se only on rep 0 (resident across reps)
F_SKEW = 4
F_DSKEW = 3          # diagonal denominator matmuls trail by this many items
F_EXP2 = False       # paired fulls: 2-bank pscr + one merged exp per pair
F_OUT_BEFORE = False # emit out_stage(qb-1) before attn_stage(qb)
F_ESBUFS = 8
F_OBG_SPLIT = False  # alternate obg copies between DVE and ACT
F_QUAD_N = 12        # fulls per denominator ones-matmul
F_OUT_ILV = True     # interleave out_stage(qb-1) matmuls into attn items


def _build(reps=1, bench_outs=None):
    nc = bacc.Bacc("TRN2", target_bir_lowering=False, debug=False, num_devices=8)

    xt_d = nc.dram_tensor("xt", [DIM, S], BF, kind="ExternalInput").ap()
    wq_d = nc.dram_tensor("wq", [DIM, LQH * HD], BF, kind="ExternalInput").ap()
    wk_d = nc.dram_tensor("wk", [DIM, HD], BF, kind="ExternalInput").ap()
    wv_d = nc.dram_tensor("wv", [DIM, HD], BF, kind="ExternalInput").ap()
    wo_d = nc.dram_tensor("wo", [LQH * HD, DIM], BF, kind="ExternalInput").ap()
    ce_d = nc.dram_tensor("ce", [HD, S], BF, kind="ExternalInput").ap()
    se_d = nc.dram_tensor("se", [HD, S], BF, kind="ExternalInput").ap()
    psw_d = (None if F_ROPE_SHUF else
             nc.dram_tensor("psw", [HD, HD], BF, kind="ExternalInput").ap())
    n_outs = bench_outs if bench_outs is not None else reps
    assert reps <= n_outs
    out_ds = [
        nc.dram_tensor("out" if r == 0 else f"out{r}", [DIM, S], BF,
                       kind="ExternalOutput").ap()
        for r in range(n_outs)
    ]

    with tile.TileContext(nc, trace_sim=False) as tc, ExitStack() as ctx:
        persist = ctx.enter_context(tc.tile_pool(name="persist", bufs=1))
        xt_pool = ctx.enter_context(tc.tile_pool(name="xtq", bufs=3))
        work = ctx.enter_context(tc.tile_pool(name="work", bufs=3))
        expp = ctx.enter_context(tc.tile_pool(name="expp", bufs=12))
        outp = ctx.enter_context(tc.tile_pool(name="outp", bufs=4))
        # PSUM budget (8 banks total): pacc 2 + pscr 4 + pmix 1 + pden 1
        psum = ctx.enter_context(tc.tile_pool(name="psum", bufs=1, space="PSUM"))

        wq_sb = persist.tile([128, NDT * LQH * HD], BF, tag="wq")   # dt-major blocks of 512
        wk_sb = persist.tile([128, NDT * HD], BF, tag="wk")
        wv_sb = persist.tile([128, NDT * HD], BF, tag="wv")
        wo_sb = persist.tile([128, LQH * DIM], BF, tag="wo")        # h-major blocks of 2048
        ce_sb = persist.tile([128, S], BF, tag="ce")
        se_sb = persist.tile([128, S], BF, tag="se")
        ones_sb = persist.tile([128, 128], BF, tag="ones")
        ident_sb = persist.tile([128, 128], BF, tag="ident")
        if F_ROPE_SHUF:
            psw_sb = None
        else:
            psw_sb = persist.tile([128, 128], BF, tag="psw")

        for _rep in range(reps):
          out_d = out_ds[_rep]
          x_src = xt_d if _rep == 0 else out_ds[_rep - 1]

          qt_sb = persist.tile([128, LQH * S], BF, tag="qt")          # roped Q^T per head
          kt_sb = persist.tile([128, S], BF, tag="kt")                # roped K^T
          vt_sb = persist.tile([128, S], BF, tag="vt")                # V^T
          vn_sb = persist.tile([128, S], BF, tag="vn")                # V natural, 16 x [128,128]
          ot_sb = persist.tile([128, LQH * S], BF, tag="ot")          # normalized attn out^T

          def dma(out_ap, in_ap):
              nc.sync.dma_start(out_ap, in_ap)

          def dma_rows(sb_ap, dram_ap, groups, cols):
              # one DMA for `groups` consecutive 128-row blocks of a row-major
              # DRAM matrix into column-blocks of a [128, groups*cols] SBUF tile
              dma(
                  sb_ap.rearrange("p (t q) -> p t q", t=groups),
                  dram_ap.rearrange("(t p) q -> p t q", p=128),
              )

          ldw = (_rep == 0) or not F_WONCE
          xq0 = xt_pool.tile([128, NDT * QB], BF, tag="xq")
          dma_rows(xq0[:, 0:QB], x_src[0:128, 0:QB], 1, QB)
          if ldw:
              dma_rows(wq_sb[:, 0:512], wq_d[0:128, :], 1, 512)
          dma_rows(xq0[:, QB:4 * QB], x_src[128:512, 0:QB], 3, QB)
          if ldw:
              dma_rows(wq_sb[:, 512:4 * 512], wq_d[128:512, :], 3, 512)
          for g in range(1, 4):
              dma_rows(
                  xq0[:, g * 4 * QB:(g + 1) * 4 * QB],
                  x_src[g * 512:(g + 1) * 512, 0:QB], 4, QB,
              )
              if ldw:
                  dma_rows(
                      wq_sb[:, g * 4 * 512:(g + 1) * 4 * 512],
                      wq_d[g * 512:(g + 1) * 512, :], 4, 512,
                  )
          if ldw:
              dma_rows(wk_sb[:], wk_d[:], NDT, HD)
              dma_rows(wv_sb[:], wv_d[:], NDT, HD)
              dma(ce_sb[:], ce_d[:])
              dma(se_sb[:], se_d[:])
              if not F_ROPE_SHUF:
                  dma(psw_sb[:], psw_d[:])
              nc.gpsimd.memset(ones_sb[:], 1.0)
              masks.make_identity(nc, ident_sb[:])

          def rope(psum_in, out_ap, qb):
              if F_ROPE_SHUF:
                  sw = work.tile([128, QB], F32, tag="sw")
                  nc.vector.stream_shuffle(sw[:], psum_in[:], SWAP_MASK)
                  t1 = work.tile([128, QB], F32, tag="t1")
                  nc.vector.tensor_mul(t1[:], psum_in[:],
                                       ce_sb[:, qb * QB:(qb + 1) * QB])
                  t2 = work.tile([128, QB], F32, tag="t2")
                  nc.vector.tensor_mul(t2[:], sw[:],
                                       se_sb[:, qb * QB:(qb + 1) * QB])
                  nc.vector.tensor_add(out_ap, t1[:], t2[:])
              else:
                  pre = work.tile([128, QB], BF, tag="pre")
                  nc.scalar.copy(pre[:], psum_in[:])
                  pu = psum.tile([128, QB], F32, tag="pmix", bufs=1)
                  nc.tensor.matmul(pu[:], psw_sb[:], pre[:], start=True, stop=True)
                  t1 = work.tile([128, QB], F32, tag="t1")
                  nc.vector.tensor_mul(t1[:], pre[:],
                                       ce_sb[:, qb * QB:(qb + 1) * QB])
                  t2 = work.tile([128, QB], F32, tag="t2")
                  nc.vector.tensor_mul(t2[:], pu[:],
                                       se_sb[:, qb * QB:(qb + 1) * QB])
                  nc.vector.tensor_add(out_ap, t1[:], t2[:])

          # ---- per-qb pipeline: proj(qb) -> attn(qb) -> out-proj(qb) ----
          def proj_stage(qb, xq):
              for h in range(LQH):
                  pq = psum.tile([128, QB], F32, tag="pacc", bufs=2)
                  for dt in range(NDT):
                      nc.tensor.matmul(
                          pq[:],
                          wq_sb[:, dt * 512 + h * 128: dt * 512 + (h + 1) * 128],
                          xq[:, dt * QB:(dt + 1) * QB],
                          start=(dt == 0),
                          stop=(dt == NDT - 1),
                      )
                  rope(pq, qt_sb[:, h * S + qb * QB: h * S + (qb + 1) * QB], qb)
              pk = psum.tile([128, QB], F32, tag="pacc", bufs=2)
              for dt in range(NDT):
                  nc.tensor.matmul(
                      pk[:],
                      wk_sb[:, dt * 128:(dt + 1) * 128],
                      xq[:, dt * QB:(dt + 1) * QB],
                      start=(dt == 0),
                      stop=(dt == NDT - 1),
                  )
              rope(pk, kt_sb[:, qb * QB:(qb + 1) * QB], qb)
              pv = psum.tile([128, QB], F32, tag="pacc", bufs=2)
              for dt in range(NDT):
                  nc.tensor.matmul(
                      pv[:],
                      wv_sb[:, dt * 128:(dt + 1) * 128],
                      xq[:, dt * QB:(dt + 1) * QB],
                      start=(dt == 0),
                      stop=(dt == NDT - 1),
                  )
              nc.scalar.copy(vt_sb[:, qb * QB:(qb + 1) * QB], pv[:])
              for kt in range(4 * qb, 4 * qb + 4):
                  ptr = psum.tile([128, 128], BF, tag="pmix", bufs=1)
                  nc.tensor.transpose(
                      ptr[:], vt_sb[:, kt * 128:(kt + 1) * 128], ident_sb[:]
                  )
                  nc.scalar.copy(vn_sb[:, kt * 128:(kt + 1) * 128], ptr[:])

          def attn_items(qb):
              # returns list of (kt, q_off, q_w, mask_mi, first, last)
              # mask_mi: None = no mask; for DIAG128 mi is True; for 256-halves
              # mi in (0, 1) selects the affine_select base offset.
              if F_DIAG128:
                  if qb == 0:
                      return [
                          (j, 128 * r, 128, True if j == r else None,
                           j == 0, j == r)
                          for r in range(4) for j in range(r + 1)
                      ]
                  items = [
                      (4 * qb + j, 128 * j, QB - 128 * j,
                       True, j == 0, False)
                      for j in range(4)
                  ]
                  items += [
                      (kt, 0, QB, None, False, kt == 4 * qb - 1)
                      for kt in range(4 * qb)
                  ]
                  return items
              raw = [(kt, 0, QB, None) for kt in range(4 * qb)]
              raw += [
                  (4 * qb, 0, 256, 0),
                  (4 * qb + 1, 0, 256, 1),
                  (4 * qb, 256, 256, None),
                  (4 * qb + 1, 256, 256, None),
                  (4 * qb + 2, 256, 256, 0),
                  (4 * qb + 3, 256, 256, 1),
              ]
              items = []
              for idx, (kt, q_off, q_w, mi) in enumerate(raw):
                  halves = range(q_off // 256, (q_off + q_w) // 256)
                  first = all(
                      not any(o2 // 256 <= hh < (o2 + w2) // 256
                              for (_, o2, w2, _) in raw[:idx])
                      for hh in halves)
                  last = all(
                      not any(o2 // 256 <= hh < (o2 + w2) // 256
                              for (_, o2, w2, _) in raw[idx + 1:])
                      for hh in halves)
                  items.append((kt, q_off, q_w, mi, first, last))
              return items

          def attn_stage(qb, outg=None):
              items = attn_items(qb)
              for h in range(LQH):
                  po = psum.tile([128, QB], F32, tag="pacc", bufs=2)
                  reg_es, quad_acc = {}, [None, None]
                  if F_DEN_DVE:
                      den = work.tile([128, QB], F32, tag="den")
                      pden_acc = None
                  else:
                      den = None
                      pden_acc = psum.tile([128, QB], F32, tag="pden",
                                           bufs=1 if (F_DEN_QUAD and F_EXP2)
                                           else 2)
                  SKEW = F_SKEW  # PV matmuls trail the score/exp stream
                  den_q = []     # deferred denominator matmul emitters
                  pden_started = [False]

                  def pden_start():
                      s = not pden_started[0]
                      pden_started[0] = True
                      return s

                  def flush_den(idx_now):
                      while den_q and den_q[0][0] <= idx_now:
                          den_q.pop(0)[1]()

                  def pv_mm(idx, es_ap):
                      kt, q_off, q_w, _, first, last = items[idx]
                      nc.tensor.matmul(
                          po[:, q_off:q_off + q_w],
                          vn_sb[:, kt * 128:(kt + 1) * 128],
                          es_ap[:, :q_w],
                          start=first,
                          stop=last,
                      )
                      if not F_DEN_DVE and not F_DEN_QUAD:
                          nc.tensor.matmul(
                              pden_acc[:, q_off:q_off + q_w],
                              ones_sb[:],
                              es_ap[:, :q_w],
                              start=first,
                              stop=last,
                          )

                  def quad_note(idx, pair_ap, group):
                      # accumulate `group` tiles into one ones-matmul; pair_ap
                      # is bf16 [128, QB] (an es tile, or a pair sum under
                      # F_EXP2 where group counts pairs)
                      n = quad_acc[1]
                      if n is None:
                          quad_acc[0] = pair_ap
                          quad_acc[1] = 1
                      elif n == 1:
                          t = expp.tile([128, QB], BF, tag="esp", bufs=4)
                          nc.vector.tensor_add(t[:], quad_acc[0][:, :QB],
                                               pair_ap[:, :QB])
                          quad_acc[0] = t
                          quad_acc[1] = 2
                      else:
                          nc.vector.tensor_add(quad_acc[0][:, :QB],
                                               quad_acc[0][:, :QB],
                                               pair_ap[:, :QB])
                          quad_acc[1] = n + 1
                      last = idx == len(items) - 1
                      if quad_acc[1] >= group or last:
                          nc.tensor.matmul(
                              pden_acc[:], ones_sb[:], quad_acc[0][:, :QB],
                              start=pden_start(),
                              stop=last,
                          )
                          quad_acc[0] = None
                          quad_acc[1] = None

                  es_ring = {}
                  pair_state = [None]  # pscr2 tile of the pending even member
                  for idx, (kt, q_off, q_w, mi, first, last) in enumerate(items):
                      paired = F_EXP2 and qb > 0 and idx >= 4
                      if F_EXP2:
                          if paired and (idx - 4) % 2 == 1:
                              pscr = pair_state[0]
                              half = 1
                          else:
                              pscr = psum.tile([128, 2 * QB], F32, tag="pscr",
                                               bufs=2)
                              pair_state[0] = pscr
                              half = 0
                          off = half * QB
                      else:
                          pscr = psum.tile([128, QB], F32, tag="pscr",
                                           bufs=F_PSCR)
                          off = 0
                      nc.tensor.matmul(
                          pscr[:, off:off + q_w],
                          kt_sb[:, kt * 128:(kt + 1) * 128],
                          qt_sb[:, h * S + qb * QB + q_off:
                                h * S + qb * QB + q_off + q_w],
                          start=True,
                          stop=True,
                      )
                      if paired and half == 0:
                          es_ring[idx] = ('pend', pscr)
                          flush_den(idx)
                          if idx >= SKEW:
                              e = es_ring.pop(idx - SKEW)
                              pv_mm(idx - SKEW, e)
                          if outg is not None:
                              next(outg, None)
                          continue
                      if paired:
                          es2 = expp.tile([128, 2 * QB], BF, tag="es2", bufs=6)
                          nc.scalar.activation(
                              es2[:, :2 * QB], pscr[:, :2 * QB],
                              mybir.ActivationFunctionType.Exp, scale=SCALE,
                          )
                          es_ring[idx - 1] = es2[:, 0:QB]
                          es_ring[idx] = es2[:, QB:2 * QB]
                          # pair sum for the denominator quad
                          t = expp.tile([128, QB], BF, tag="esp", bufs=4)
                          nc.vector.tensor_add(t[:], es2[:, 0:QB],
                                               es2[:, QB:2 * QB])
                          quad_note(idx, t, F_QUAD_N // 2)
                      else:
                          es = expp.tile([128, QB], BF, tag="es",
                                         bufs=F_ESBUFS)
                          nc.scalar.activation(
                              es[:, :q_w], pscr[:, off:off + q_w],
                              mybir.ActivationFunctionType.Exp, scale=SCALE,
                          )
                          if mi is not None:
                              if F_DIAG128:
                                  nc.gpsimd.affine_select(
                                      out=es[:, :128],
                                      in_=es[:, :128],
                                      compare_op=mybir.AluOpType.is_ge,
                                      fill=0.0,
                                      base=0,
                                      channel_multiplier=-1,
                                      pattern=[[1, 128]],
                                  )
                              else:
                                  nc.gpsimd.affine_select(
                                      out=es[:, :q_w],
                                      in_=es[:, :q_w],
                                      compare_op=mybir.AluOpType.is_ge,
                                      fill=0.0,
                                      base=-128 * mi,
                                      channel_multiplier=-1,
                                      pattern=[[1, q_w]],
                                  )
                          if F_DEN_DVE:
                              deng = nc.vector if F_DEN_ENG == 'v' else nc.gpsimd
                              if first:
                                  deng.tensor_copy(
                                      den[:, q_off:q_off + q_w], es[:, :q_w]
                                  )
                              else:
                                  deng.tensor_add(
                                      den[:, q_off:q_off + q_w],
                                      den[:, q_off:q_off + q_w],
                                      es[:, :q_w],
                                  )
                          if F_DEN_QUAD:
                              if qb == 0:
                                  r = q_off // 128
                                  if kt == 0:
                                      reg_es[r] = es
                                  elif kt == 1:
                                      t = expp.tile([128, 128], BF, tag="esr",
                                                    bufs=4)
                                      nc.vector.tensor_add(
                                          t[:], reg_es[r][:, :128],
                                          es[:, :128])
                                      reg_es[r] = t
                                  else:
                                      nc.vector.tensor_add(
                                          reg_es[r][:, :128],
                                          reg_es[r][:, :128], es[:, :128])
                                  if kt == r:
                                      def emit_reg(r=r, q_off=q_off):
                                          nc.tensor.matmul(
                                              pden_acc[:, q_off:q_off + 128],
                                              ones_sb[:],
                                              reg_es[r][:, :128],
                                              start=True, stop=True,
                                          )
                                      den_q.append((idx + F_DSKEW, emit_reg))
                              elif idx < 4:  # diagonal
                                  def emit_diag(q_off=q_off, q_w=q_w, es=es):
                                      nc.tensor.matmul(
                                          pden_acc[:, q_off:q_off + q_w],
                                          ones_sb[:], es[:, :q_w],
                                          start=pden_start(),
                                          stop=False,
                                      )
                                  if F_DSKEW == 0:
                                      emit_diag()
                                  else:
                                      den_q.append((idx + F_DSKEW, emit_diag))
                              else:  # fulls, unpaired mode
                                  quad_note(idx, es, F_QUAD_N)
                          es_ring[idx] = es
                      flush_den(idx)
                      if idx >= SKEW:
                          e = es_ring.pop(idx - SKEW)
                          pv_mm(idx - SKEW, e)
                      if outg is not None:
                          next(outg, None)
                  while den_q:
                      den_q.pop(0)[1]()
                  for idx in range(max(0, len(items) - SKEW), len(items)):
                      pv_mm(idx, es_ring.pop(idx))
                  if F_DEN_DVE:
                      denb = work.tile([128, QB], BF, tag="denb")
                      deng2 = nc.vector if F_DEN_ENG == 'v' else nc.gpsimd
                      deng2.tensor_copy(denb[:], den[:])
                      pden = psum.tile([128, QB], F32, tag="pden", bufs=1)
                      nc.tensor.matmul(
                          pden[:], ones_sb[:], denb[:], start=True, stop=True
                      )
                  else:
                      pden = pden_acc
                  rec = work.tile([128, QB], F32, tag="rec")
                  nc.vector.reciprocal(rec[:], pden[:])
                  nc.vector.tensor_mul(
                      ot_sb[:, h * S + qb * QB: h * S + (qb + 1) * QB], po[:], rec[:]
                  )

          _pscr_b = 2 if F_EXP2 else F_PSCR
          _pden_b = (1 if (F_DEN_QUAD and F_EXP2) else 2) if not F_DEN_DVE else 1
          _p3tags = [("pacc", 2), ("pscr", _pscr_b), ("pmix", 1),
                     ("pden", _pden_b)]

          def out_stage_gen(qb, ilv):
              # generator: when ilv, yields after each matmul so the caller
              # can interleave out-proj work into the attention item stream
              for grp in range(4):
                  obg = outp.tile([128, 4 * QB], BF, tag="obg")
                  for i in range(4):
                      dct = grp * 4 + i
                      if ilv:
                          pw = psum.tile([128, QB], F32, tag="pmix", bufs=1)
                      else:
                          _t, _b = _p3tags[dct % 4]
                          pw = psum.tile([128, QB], F32, tag=_t, bufs=_b)
                      for h in range(LQH):
                          nc.tensor.matmul(
                              pw[:],
                              wo_sb[:, h * DIM + dct * 128: h * DIM + (dct + 1) * 128],
                              ot_sb[:, h * S + qb * QB: h * S + (qb + 1) * QB],
                              start=(h == 0),
                              stop=(h == LQH - 1),
                          )
                          yield
                      if F_OBG_SPLIT and i % 2 == 1:
                          nc.scalar.copy(obg[:, i * QB:(i + 1) * QB], pw[:])
                      else:
                          nc.vector.tensor_copy(obg[:, i * QB:(i + 1) * QB], pw[:])
                  dma(
                      out_d[grp * 512:(grp + 1) * 512,
                            qb * QB:(qb + 1) * QB].rearrange(
                          "(t p) q -> p t q", p=128),
                      obg.rearrange("p (t q) -> p t q", t=4),
                  )

          def out_stage(qb):
              for _ in out_stage_gen(qb, False):
                  pass

          xqs = {0: xq0}
          for qb in range(NQB):
              proj_stage(qb, xqs.pop(qb))
              if qb + 1 < NQB:  # prefetch next x block before attention fills time
                  xq = xt_pool.tile([128, NDT * QB], BF, tag="xq")
                  for g in range(4):
                      dma_rows(
                          xq[:, g * 4 * QB:(g + 1) * 4 * QB],
                          x_src[g * 512:(g + 1) * 512, (qb + 1) * QB:(qb + 2) * QB],
                          4, QB,
                      )
                  xqs[qb + 1] = xq
              if qb == 0 and ldw:  # wo lands during attention(0)
                  for h in range(LQH):
                      dma(
                          wo_sb[:, h * DIM:(h + 1) * DIM],
                          wo_d[h * 128:(h + 1) * 128, :],
                      )
              if F_OUT_ILV and qb > 0:
                  outg = out_stage_gen(qb - 1, True)
                  attn_stage(qb, outg)
                  for _ in outg:  # drain leftovers
                      pass
              elif F_OUT_BEFORE:
                  if qb > 0:
                      out_stage(qb - 1)
                  attn_stage(qb)
              else:
                  attn_stage(qb)
                  if qb > 0:
                      out_stage(qb - 1)
          out_stage(NQB - 1)

    nc.compile()
    return nc


_NC_CACHE = []


def _get_nc():
    if not _NC_CACHE:
        _NC_CACHE.append(_build())
    return _NC_CACHE[0]


def _make_in_maps(x, cos, sin, wq, wk, wv, wo):
    bf = ml_dtypes.bfloat16
    ce = np.repeat(np.ascontiguousarray(cos.T), 2, axis=0).astype(bf)  # [128, S]
    se = np.repeat(np.ascontiguousarray(sin.T), 2, axis=0)
    se[0::2] *= -1.0  # sign of the pair swap folded in: out[2p] -= in[2p+1]*sin
    se = se.astype(bf)
    xt = [np.ascontiguousarray(x[b].T).astype(bf) for b in range(B)]
    in_maps = []
    for c in range(8):
        b, r = divmod(c, TPR)
        in_maps.append(
            {
                "xt": xt[b],
                "wq": np.ascontiguousarray(wq[:, r * 512:(r + 1) * 512]).astype(bf),
                "wk": np.ascontiguousarray(wk[:, r * 128:(r + 1) * 128]).astype(bf),
                "wv": np.ascontiguousarray(wv[:, r * 128:(r + 1) * 128]).astype(bf),
                "wo": np.ascontiguousarray(wo[r * 512:(r + 1) * 512, :]).astype(bf),
                "ce": ce,
                "se": se,
            }
        )
    return in_maps


def _assemble(results):
    full = np.empty((B, S, DIM), np.float32)
    for b in range(B):
        acc = results[TPR * b]["out"].astype(np.float32)
        for r in range(1, TPR):
            acc += results[TPR * b + r]["out"].astype(np.float32)
        full[b] = acc.T
    return full


def kernel(x, cos, sin, wq, wk, wv, wo):
    x = np.asarray(x, np.float32)
    cos = np.asarray(cos, np.float32)
    sin = np.asarray(sin, np.float32)
    wq = np.asarray(wq, np.float32)
    wk = np.asarray(wk, np.float32)
    wv = np.asarray(wv, np.float32)
    wo = np.asarray(wo, np.float32)

    nc = _get_nc()
    in_maps = _make_in_maps(x, cos, sin, wq, wk, wv, wo)
    res = bass_utils.run_bass_kernel_spmd(nc, in_maps, core_ids=list(range(8)))
    return _assemble(res.results)


def run_traced(inputs):
    """Timing/profiling helper for test.py (not used by the grader)."""
    nc = _get_nc()
    in_maps = _make_in_maps(
        np.asarray(inputs["x"], np.float32),
        np.asarray(inputs["cos"], np.float32),
        np.asarray(inputs["sin"], np.float32),
        np.asarray(inputs["wq"], np.float32),
        np.asarray(inputs["wk"], np.float32),
        np.asarray(inputs["wv"], np.float32),
        np.asarray(inputs["wo"], np.float32),
    )
    res = bass_utils.run_bass_kernel_spmd(
        nc, in_maps, core_ids=list(range(8)), trace=True
    )
    return res


# revision 5
# speedup vs baseline: 1.9724x; 1.9724x over previous
"""GQA attention (B=2, S=2048, 16 q heads / 4 kv heads, head_dim=128) with RoPE
and causal softmax, tensor-parallel over heads x data-parallel over batch on
8 Trainium2 NeuronCores.

Core c (0..7): batch b = c//4, TP rank r = c%4.
Each core computes, for its batch and its 4 q heads / 1 kv head:
  QT/KT/VT projections (transposed layout, contraction on partitions),
  RoPE via a DVE partition pair-swap (stream_shuffle) + elementwise muls
  (sign of the swap folded into the host-prepared sin table),
  causal softmax without max-subtraction (scores bounded; exp in fp32->bf16),
  softmax denominator via DVE accumulation of exp tiles across key tiles
  + one ones-matmul per (head, q-block) for the partition reduction,
  P@V in transposed layout (no transposes of the probability tiles),
  causal diagonal handled at 128-column granularity,
  row-sharded output projection producing a partial [DIM, S] f32 output.
Host sums the 4 partials per batch (the row-parallel all-reduce) + transposes.
"""

import numpy as np
import ml_dtypes
from contextlib import ExitStack

import concourse.bass as bass
import concourse.tile as tile
from concourse import bacc, mybir, bass_utils, masks

B, S, DIM = 2, 2048, 2048
NH, NKV, HD = 16, 4, 128
TPR = 4            # tensor-parallel ranks per batch
LQH = NH // TPR    # 4 local q heads
QB = 512           # q block (free dim of matmuls)
NQB = S // QB      # 4
NDT = DIM // 128   # 16 contraction tiles for the projections
NKT = S // 128     # 16 key tiles
SCALE = 1.0 / float(np.sqrt(HD))
SWAP_MASK = [i ^ 1 for i in range(32)]  # partition pair swap within quadrants

BF = mybir.dt.bfloat16
F32 = mybir.dt.float32

# schedule/config constants (tuned via TimelineSim cost model)
F_ROPE_SHUF = True   # RoPE via DVE stream_shuffle (else psw matmul)
F_DIAG128 = True     # 128-col diagonal granularity (else 256 halves)
F_DEN_DVE = False    # denominator via full DVE accumulation (DVE-bound: off)
F_DEN_ENG = 'v'      # 'v' = vector engine, 'g' = gpsimd for den accumulation
F_DEN_QUAD = True    # denominator: bf16 group-accumulate fulls on DVE + 1 matmul/group
F_PSCR = 3
F_WONCE = True       # load weights/ce/se only on rep 0 (resident across reps)
F_SKEW = 4
F_DSKEW = 8          # diag denominator matmuls trail by this many items (clamped)
F_EXP2 = False       # paired fulls: 2-bank pscr + one merged exp per pair
F_OUT_BEFORE = False # emit out_stage(qb-1) before attn_stage(qb)
F_ESBUFS = 12
F_OBG_SPLIT = True   # alternate obg copies between DVE and ACT
F_QUAD_N = 12        # fulls per denominator ones-matmul
F_OUT_ILV = True     # interleave out_stage(qb-1) matmuls into attn items


def _build(reps=1, bench_outs=None):
    nc = bacc.Bacc("TRN2", target_bir_lowering=False, debug=False, num_devices=8)

    xt_d = nc.dram_tensor("xt", [DIM, S], BF, kind="ExternalInput").ap()
    wq_d = nc.dram_tensor("wq", [DIM, LQH * HD], BF, kind="ExternalInput").ap()
    wk_d = nc.dram_tensor("wk", [DIM, HD], BF, kind="ExternalInput").ap()
    wv_d = nc.dram_tensor("wv", [DIM, HD], BF, kind="ExternalInput").ap()
    wo_d = nc.dram_tensor("wo", [LQH * HD, DIM], BF, kind="ExternalInput").ap()
    ce_d = nc.dram_tensor("ce", [HD, S], BF, kind="ExternalInput").ap()
    se_d = nc.dram_tensor("se", [HD, S], BF, kind="ExternalInput").ap()
    psw_d = (None if F_ROPE_SHUF else
             nc.dram_tensor("psw", [HD, HD], BF, kind="ExternalInput").ap())
    n_outs = bench_outs if bench_outs is not None else reps
    assert reps <= n_outs
    out_ds = [
        nc.dram_tensor("out" if r == 0 else f"out{r}", [DIM, S], BF,
                       kind="ExternalOutput").ap()
        for r in range(n_outs)
    ]

    with tile.TileContext(nc, trace_sim=False) as tc, ExitStack() as ctx:
        persist = ctx.enter_context(tc.tile_pool(name="persist", bufs=1))
        xt_pool = ctx.enter_context(tc.tile_pool(name="xtq", bufs=3))
        work = ctx.enter_context(tc.tile_pool(name="work", bufs=3))
        expp = ctx.enter_context(tc.tile_pool(name="expp", bufs=12))
        outp = ctx.enter_context(tc.tile_pool(name="outp", bufs=4))
        # PSUM budget (8 banks total): pacc 2 + pscr 4 + pmix 1 + pden 1
        psum = ctx.enter_context(tc.tile_pool(name="psum", bufs=1, space="PSUM"))

        wq_sb = persist.tile([128, NDT * LQH * HD], BF, tag="wq")   # dt-major blocks of 512
        wk_sb = persist.tile([128, NDT * HD], BF, tag="wk")
        wv_sb = persist.tile([128, NDT * HD], BF, tag="wv")
        wo_sb = persist.tile([128, LQH * DIM], BF, tag="wo")        # h-major blocks of 2048
        ce_sb = persist.tile([128, S], BF, tag="ce")
        se_sb = persist.tile([128, S], BF, tag="se")
        ones_sb = persist.tile([128, 128], BF, tag="ones")
        ident_sb = persist.tile([128, 128], BF, tag="ident")
        if F_ROPE_SHUF:
            psw_sb = None
        else:
            psw_sb = persist.tile([128, 128], BF, tag="psw")

        for _rep in range(reps):
          out_d = out_ds[_rep]
          x_src = xt_d if _rep == 0 else out_ds[_rep - 1]

          qt_sb = persist.tile([128, LQH * S], BF, tag="qt")          # roped Q^T per head
          kt_sb = persist.tile([128, S], BF, tag="kt")                # roped K^T
          vt_sb = persist.tile([128, S], BF, tag="vt")                # V^T
          vn_sb = persist.tile([128, S], BF, tag="vn")                # V natural, 16 x [128,128]
          ot_sb = persist.tile([128, LQH * S], BF, tag="ot")          # normalized attn out^T

          def dma(out_ap, in_ap):
              nc.sync.dma_start(out_ap, in_ap)

          def dma_rows(sb_ap, dram_ap, groups, cols):
              # one DMA for `groups` consecutive 128-row blocks of a row-major
              # DRAM matrix into column-blocks of a [128, groups*cols] SBUF tile
              dma(
                  sb_ap.rearrange("p (t q) -> p t q", t=groups),
                  dram_ap.rearrange("(t p) q -> p t q", p=128),
              )

          ldw = (_rep == 0) or not F_WONCE
          xq0 = xt_pool.tile([128, NDT * QB], BF, tag="xq")
          dma_rows(xq0[:, 0:QB], x_src[0:128, 0:QB], 1, QB)
          if ldw:
              dma_rows(wq_sb[:, 0:512], wq_d[0:128, :], 1, 512)
          dma_rows(xq0[:, QB:4 * QB], x_src[128:512, 0:QB], 3, QB)
          if ldw:
              dma_rows(wq_sb[:, 512:4 * 512], wq_d[128:512, :], 3, 512)
          for g in range(1, 4):
              dma_rows(
                  xq0[:, g * 4 * QB:(g + 1) * 4 * QB],
                  x_src[g * 512:(g + 1) * 512, 0:QB], 4, QB,
              )
              if ldw:
                  dma_rows(
                      wq_sb[:, g * 4 * 512:(g + 1) * 4 * 512],
                      wq_d[g * 512:(g + 1) * 512, :], 4, 512,
                  )
          if ldw:
              dma_rows(wk_sb[:], wk_d[:], NDT, HD)
              dma_rows(wv_sb[:], wv_d[:], NDT, HD)
              dma(ce_sb[:], ce_d[:])
              dma(se_sb[:], se_d[:])
              if not F_ROPE_SHUF:
                  dma(psw_sb[:], psw_d[:])
              nc.gpsimd.memset(ones_sb[:], 1.0)
              masks.make_identity(nc, ident_sb[:])

          def rope(psum_in, out_ap, qb):
              if F_ROPE_SHUF:
                  sw = work.tile([128, QB], F32, tag="sw")
                  nc.vector.stream_shuffle(sw[:], psum_in[:], SWAP_MASK)
                  t1 = work.tile([128, QB], F32, tag="t1")
                  nc.vector.tensor_mul(t1[:], psum_in[:],
                                       ce_sb[:, qb * QB:(qb + 1) * QB])
                  t2 = work.tile([128, QB], F32, tag="t2")
                  nc.vector.tensor_mul(t2[:], sw[:],
                                       se_sb[:, qb * QB:(qb + 1) * QB])
                  nc.vector.tensor_add(out_ap, t1[:], t2[:])
              else:
                  pre = work.tile([128, QB], BF, tag="pre")
                  nc.scalar.copy(pre[:], psum_in[:])
                  pu = psum.tile([128, QB], F32, tag="pmix", bufs=1)
                  nc.tensor.matmul(pu[:], psw_sb[:], pre[:], start=True, stop=True)
                  t1 = work.tile([128, QB], F32, tag="t1")
                  nc.vector.tensor_mul(t1[:], pre[:],
                                       ce_sb[:, qb * QB:(qb + 1) * QB])
                  t2 = work.tile([128, QB], F32, tag="t2")
                  nc.vector.tensor_mul(t2[:], pu[:],
                                       se_sb[:, qb * QB:(qb + 1) * QB])
                  nc.vector.tensor_add(out_ap, t1[:], t2[:])

          # ---- per-qb pipeline: proj(qb) -> attn(qb) -> out-proj(qb) ----
          def proj_stage(qb, xq):
              for h in range(LQH):
                  pq = psum.tile([128, QB], F32, tag="pacc", bufs=2)
                  for dt in range(NDT):
                      nc.tensor.matmul(
                          pq[:],
                          wq_sb[:, dt * 512 + h * 128: dt * 512 + (h + 1) * 128],
                          xq[:, dt * QB:(dt + 1) * QB],
                          start=(dt == 0),
                          stop=(dt == NDT - 1),
                      )
                  rope(pq, qt_sb[:, h * S + qb * QB: h * S + (qb + 1) * QB], qb)
              pk = psum.tile([128, QB], F32, tag="pacc", bufs=2)
              for dt in range(NDT):
                  nc.tensor.matmul(
                      pk[:],
                      wk_sb[:, dt * 128:(dt + 1) * 128],
                      xq[:, dt * QB:(dt + 1) * QB],
                      start=(dt == 0),
                      stop=(dt == NDT - 1),
                  )
              rope(pk, kt_sb[:, qb * QB:(qb + 1) * QB], qb)
              pv = psum.tile([128, QB], F32, tag="pacc", bufs=2)
              for dt in range(NDT):
                  nc.tensor.matmul(
                      pv[:],
                      wv_sb[:, dt * 128:(dt + 1) * 128],
                      xq[:, dt * QB:(dt + 1) * QB],
                      start=(dt == 0),
                      stop=(dt == NDT - 1),
                  )
              nc.scalar.copy(vt_sb[:, qb * QB:(qb + 1) * QB], pv[:])
              for kt in range(4 * qb, 4 * qb + 4):
                  ptr = psum.tile([128, 128], BF, tag="pmix", bufs=1)
                  nc.tensor.transpose(
                      ptr[:], vt_sb[:, kt * 128:(kt + 1) * 128], ident_sb[:]
                  )
                  nc.scalar.copy(vn_sb[:, kt * 128:(kt + 1) * 128], ptr[:])

          def attn_items(qb):
              # returns list of (kt, q_off, q_w, mask_mi, first, last)
              # mask_mi: None = no mask; for DIAG128 mi is True; for 256-halves
              # mi in (0, 1) selects the affine_select base offset.
              if F_DIAG128:
                  if qb == 0:
                      return [
                          (j, 128 * r, 128, True if j == r else None,
                           j == 0, j == r)
                          for r in range(4) for j in range(r + 1)
                      ]
                  items = [
                      (4 * qb + j, 128 * j, QB - 128 * j,
                       True, j == 0, False)
                      for j in range(4)
                  ]
                  items += [
                      (kt, 0, QB, None, False, kt == 4 * qb - 1)
                      for kt in range(4 * qb)
                  ]
                  return items
              raw = [(kt, 0, QB, None) for kt in range(4 * qb)]
              raw += [
                  (4 * qb, 0, 256, 0),
                  (4 * qb + 1, 0, 256, 1),
                  (4 * qb, 256, 256, None),
                  (4 * qb + 1, 256, 256, None),
                  (4 * qb + 2, 256, 256, 0),
                  (4 * qb + 3, 256, 256, 1),
              ]
              items = []
              for idx, (kt, q_off, q_w, mi) in enumerate(raw):
                  halves = range(q_off // 256, (q_off + q_w) // 256)
                  first = all(
                      not any(o2 // 256 <= hh < (o2 + w2) // 256
                              for (_, o2, w2, _) in raw[:idx])
                      for hh in halves)
                  last = all(
                      not any(o2 // 256 <= hh < (o2 + w2) // 256
                              for (_, o2, w2, _) in raw[idx + 1:])
                      for hh in halves)
                  items.append((kt, q_off, q_w, mi, first, last))
              return items

          def attn_stage(qb, outg=None):
              items = attn_items(qb)
              for h in range(LQH):
                  po = psum.tile([128, QB], F32, tag="pacc", bufs=2)
                  reg_es, quad_acc = {}, [None, None]
                  if F_DEN_DVE:
                      den = work.tile([128, QB], F32, tag="den")
                      pden_acc = None
                  else:
                      den = None
                      pden_acc = psum.tile([128, QB], F32, tag="pden",
                                           bufs=1 if (F_DEN_QUAD and F_EXP2)
                                           else 2)
                  SKEW = F_SKEW  # PV matmuls trail the score/exp stream
                  den_q = []     # deferred denominator matmul emitters
                  pden_started = [False]

                  def pden_start():
                      s = not pden_started[0]
                      pden_started[0] = True
                      return s

                  def flush_den(idx_now):
                      while den_q and den_q[0][0] <= idx_now:
                          den_q.pop(0)[1]()

                  def pv_mm(idx, es_ap):
                      kt, q_off, q_w, _, first, last = items[idx]
                      nc.tensor.matmul(
                          po[:, q_off:q_off + q_w],
                          vn_sb[:, kt * 128:(kt + 1) * 128],
                          es_ap[:, :q_w],
                          start=first,
                          stop=last,
                      )
                      if not F_DEN_DVE and not F_DEN_QUAD:
                          nc.tensor.matmul(
                              pden_acc[:, q_off:q_off + q_w],
                              ones_sb[:],
                              es_ap[:, :q_w],
                              start=first,
                              stop=last,
                          )

                  def quad_note(idx, pair_ap, group):
                      # accumulate `group` tiles into one ones-matmul; pair_ap
                      # is bf16 [128, QB] (an es tile, or a pair sum under
                      # F_EXP2 where group counts pairs)
                      n = quad_acc[1]
                      if n is None:
                          quad_acc[0] = pair_ap
                          quad_acc[1] = 1
                      elif n == 1:
                          t = expp.tile([128, QB], BF, tag="esp", bufs=4)
                          nc.vector.tensor_add(t[:], quad_acc[0][:, :QB],
                                               pair_ap[:, :QB])
                          quad_acc[0] = t
                          quad_acc[1] = 2
                      else:
                          nc.vector.tensor_add(quad_acc[0][:, :QB],
                                               quad_acc[0][:, :QB],
                                               pair_ap[:, :QB])
                          quad_acc[1] = n + 1
                      last = idx == len(items) - 1
                      if quad_acc[1] >= group or last:
                          nc.tensor.matmul(
                              pden_acc[:], ones_sb[:], quad_acc[0][:, :QB],
                              start=pden_start(),
                              stop=last,
                          )
                          quad_acc[0] = None
                          quad_acc[1] = None

                  es_ring = {}
                  pair_state = [None]  # pscr2 tile of the pending even member
                  for idx, (kt, q_off, q_w, mi, first, last) in enumerate(items):
                      paired = F_EXP2 and qb > 0 and idx >= 4
                      if F_EXP2:
                          if paired and (idx - 4) % 2 == 1:
                              pscr = pair_state[0]
                              half = 1
                          else:
                              pscr = psum.tile([128, 2 * QB], F32, tag="pscr",
                                               bufs=2)
                              pair_state[0] = pscr
                              half = 0
                          off = half * QB
                      else:
                          pscr = psum.tile([128, QB], F32, tag="pscr",
                                           bufs=F_PSCR)
                          off = 0
                      nc.tensor.matmul(
                          pscr[:, off:off + q_w],
                          kt_sb[:, kt * 128:(kt + 1) * 128],
                          qt_sb[:, h * S + qb * QB + q_off:
                                h * S + qb * QB + q_off + q_w],
                          start=True,
                          stop=True,
                      )
                      if paired and half == 0:
                          es_ring[idx] = ('pend', pscr)
                          flush_den(idx)
                          if idx >= SKEW:
                              e = es_ring.pop(idx - SKEW)
                              pv_mm(idx - SKEW, e)
                          if outg is not None:
                              next(outg, None)
                          continue
                      if paired:
                          es2 = expp.tile([128, 2 * QB], BF, tag="es2", bufs=6)
                          nc.scalar.activation(
                              es2[:, :2 * QB], pscr[:, :2 * QB],
                              mybir.ActivationFunctionType.Exp, scale=SCALE,
                          )
                          es_ring[idx - 1] = es2[:, 0:QB]
                          es_ring[idx] = es2[:, QB:2 * QB]
                          # pair sum for the denominator quad
                          t = expp.tile([128, QB], BF, tag="esp", bufs=4)
                          nc.vector.tensor_add(t[:], es2[:, 0:QB],
                                               es2[:, QB:2 * QB])
                          quad_note(idx, t, F_QUAD_N // 2)
                      else:
                          es = expp.tile([128, QB], BF, tag="es",
                                         bufs=F_ESBUFS)
                          nc.scalar.activation(
                              es[:, :q_w], pscr[:, off:off + q_w],
                              mybir.ActivationFunctionType.Exp, scale=SCALE,
                          )
                          if mi is not None:
                              if F_DIAG128:
                                  nc.gpsimd.affine_select(
                                      out=es[:, :128],
                                      in_=es[:, :128],
                                      compare_op=mybir.AluOpType.is_ge,
                                      fill=0.0,
                                      base=0,
                                      channel_multiplier=-1,
                                      pattern=[[1, 128]],
                                  )
                              else:
                                  nc.gpsimd.affine_select(
                                      out=es[:, :q_w],
                                      in_=es[:, :q_w],
                                      compare_op=mybir.AluOpType.is_ge,
                                      fill=0.0,
                                      base=-128 * mi,
                                      channel_multiplier=-1,
                                      pattern=[[1, q_w]],
                                  )
                          if F_DEN_DVE:
                              deng = nc.vector if F_DEN_ENG == 'v' else nc.gpsimd
                              if first:
                                  deng.tensor_copy(
                                      den[:, q_off:q_off + q_w], es[:, :q_w]
                                  )
                              else:
                                  deng.tensor_add(
                                      den[:, q_off:q_off + q_w],
                                      den[:, q_off:q_off + q_w],
                                      es[:, :q_w],
                                  )
                          if F_DEN_QUAD:
                              if qb == 0:
                                  r = q_off // 128
                                  if kt == 0:
                                      reg_es[r] = es
                                  elif kt == 1:
                                      t = expp.tile([128, 128], BF, tag="esr",
                                                    bufs=4)
                                      nc.vector.tensor_add(
                                          t[:], reg_es[r][:, :128],
                                          es[:, :128])
                                      reg_es[r] = t
                                  else:
                                      nc.vector.tensor_add(
                                          reg_es[r][:, :128],
                                          reg_es[r][:, :128], es[:, :128])
                                  if kt == r:
                                      def emit_reg(r=r, q_off=q_off):
                                          nc.tensor.matmul(
                                              pden_acc[:, q_off:q_off + 128],
                                              ones_sb[:],
                                              reg_es[r][:, :128],
                                              start=True, stop=True,
                                          )
                                      den_q.append(
                                          (min(idx + F_DSKEW,
                                               len(items) - 2), emit_reg))
                              elif idx < 4:  # diagonal
                                  def emit_diag(q_off=q_off, q_w=q_w, es=es):
                                      nc.tensor.matmul(
                                          pden_acc[:, q_off:q_off + q_w],
                                          ones_sb[:], es[:, :q_w],
                                          start=pden_start(),
                                          stop=False,
                                      )
                                  if F_DSKEW == 0:
                                      emit_diag()
                                  else:
                                      den_q.append(
                                          (min(idx + F_DSKEW,
                                               len(items) - 2), emit_diag))
                              else:  # fulls, unpaired mode
                                  quad_note(idx, es, F_QUAD_N)
                          es_ring[idx] = es
                      flush_den(idx)
                      if idx >= SKEW:
                          e = es_ring.pop(idx - SKEW)
                          pv_mm(idx - SKEW, e)
                      if outg is not None:
                          next(outg, None)
                  while den_q:
                      den_q.pop(0)[1]()
                  for idx in range(max(0, len(items) - SKEW), len(items)):
                      pv_mm(idx, es_ring.pop(idx))
                  if F_DEN_DVE:
                      denb = work.tile([128, QB], BF, tag="denb")
                      deng2 = nc.vector if F_DEN_ENG == 'v' else nc.gpsimd
                      deng2.tensor_copy(denb[:], den[:])
                      pden = psum.tile([128, QB], F32, tag="pden", bufs=1)
                      nc.tensor.matmul(
                          pden[:], ones_sb[:], denb[:], start=True, stop=True
                      )
                  else:
                      pden = pden_acc
                  rec = work.tile([128, QB], F32, tag="rec")
                  nc.vector.reciprocal(rec[:], pden[:])
                  nc.vector.tensor_mul(
                      ot_sb[:, h * S + qb * QB: h * S + (qb + 1) * QB], po[:], rec[:]
                  )

          _pscr_b = 2 if F_EXP2 else F_PSCR
          _pden_b = (1 if (F_DEN_QUAD and F_EXP2) else 2) if not F_DEN_DVE else 1
          _p3tags = [("pacc", 2), ("pscr", _pscr_b), ("pmix", 1),
                     ("pden", _pden_b)]

          def out_stage_gen(qb, ilv):
              # generator: when ilv, yields after each matmul so the caller
              # can interleave out-proj work into the attention item stream
              for grp in range(4):
                  obg = outp.tile([128, 4 * QB], BF, tag="obg")
                  for i in range(4):
                      dct = grp * 4 + i
                      if ilv:
                          pw = psum.tile([128, QB], F32, tag="pmix", bufs=1)
                      else:
                          _t, _b = _p3tags[dct % 4]
                          pw = psum.tile([128, QB], F32, tag=_t, bufs=_b)
                      for h in range(LQH):
                          nc.tensor.matmul(
                              pw[:],
                              wo_sb[:, h * DIM + dct * 128: h * DIM + (dct + 1) * 128],
                              ot_sb[:, h * S + qb * QB: h * S + (qb + 1) * QB],
                              start=(h == 0),
                              stop=(h == LQH - 1),
                          )
                          yield
                      if F_OBG_SPLIT and i % 2 == 1:
                          nc.scalar.copy(obg[:, i * QB:(i + 1) * QB], pw[:])
                      else:
                          nc.vector.tensor_copy(obg[:, i * QB:(i + 1) * QB], pw[:])
                  dma(
                      out_d[grp * 512:(grp + 1) * 512,
                            qb * QB:(qb + 1) * QB].rearrange(
                          "(t p) q -> p t q", p=128),
                      obg.rearrange("p (t q) -> p t q", t=4),
                  )

          def out_stage(qb):
              for _ in out_stage_gen(qb, False):
                  pass

          xqs = {0: xq0}
          for qb in range(NQB):
              proj_stage(qb, xqs.pop(qb))
              if qb + 1 < NQB:  # prefetch next x block before attention fills time
                  xq = xt_pool.tile([128, NDT * QB], BF, tag="xq")
                  for g in range(4):
                      dma_rows(
                          xq[:, g * 4 * QB:(g + 1) * 4 * QB],
                          x_src[g * 512:(g + 1) * 512, (qb + 1) * QB:(qb + 2) * QB],
                          4, QB,
                      )
                  xqs[qb + 1] = xq
              if qb == 0 and ldw:  # wo lands during attention(0)
                  for h in range(LQH):
                      dma(
                          wo_sb[:, h * DIM:(h + 1) * DIM],
                          wo_d[h * 128:(h + 1) * 128, :],
                      )
              if F_OUT_ILV and qb > 0:
                  outg = out_stage_gen(qb - 1, True)
                  attn_stage(qb, outg)
                  for _ in outg:  # drain leftovers
                      pass
              elif F_OUT_BEFORE:
                  if qb > 0:
                      out_stage(qb - 1)
                  attn_stage(qb)
              else:
                  attn_stage(qb)
                  if qb > 0:
                      out_stage(qb - 1)
          out_stage(NQB - 1)

    nc.compile()
    return nc


_NC_CACHE = []


def _get_nc():
    if not _NC_CACHE:
        _NC_CACHE.append(_build())
    return _NC_CACHE[0]


def _make_in_maps(x, cos, sin, wq, wk, wv, wo):
    bf = ml_dtypes.bfloat16
    ce = np.repeat(np.ascontiguousarray(cos.T), 2, axis=0).astype(bf)  # [128, S]
    se = np.repeat(np.ascontiguousarray(sin.T), 2, axis=0)
    se[0::2] *= -1.0  # sign of the pair swap folded in: out[2p] -= in[2p+1]*sin
    se = se.astype(bf)
    xt = [np.ascontiguousarray(x[b].T).astype(bf) for b in range(B)]
    in_maps = []
    for c in range(8):
        b, r = divmod(c, TPR)
        in_maps.append(
            {
                "xt": xt[b],
                "wq": np.ascontiguousarray(wq[:, r * 512:(r + 1) * 512]).astype(bf),
                "wk": np.ascontiguousarray(wk[:, r * 128:(r + 1) * 128]).astype(bf),
                "wv": np.ascontiguousarray(wv[:, r * 128:(r + 1) * 128]).astype(bf),
                "wo": np.ascontiguousarray(wo[r * 512:(r + 1) * 512, :]).astype(bf),
                "ce": ce,
                "se": se,
            }
        )
    return in_maps


def _assemble(results):
    full = np.empty((B, S, DIM), np.float32)
    for b in range(B):
        acc = results[TPR * b]["out"].astype(np.float32)
        for r in range(1, TPR):
            acc += results[TPR * b + r]["out"].astype(np.float32)
        full[b] = acc.T
    return full


def kernel(x, cos, sin, wq, wk, wv, wo):
    x = np.asarray(x, np.float32)
    cos = np.asarray(cos, np.float32)
    sin = np.asarray(sin, np.float32)
    wq = np.asarray(wq, np.float32)
    wk = np.asarray(wk, np.float32)
    wv = np.asarray(wv, np.float32)
    wo = np.asarray(wo, np.float32)

    nc = _get_nc()
    in_maps = _make_in_maps(x, cos, sin, wq, wk, wv, wo)
    res = bass_utils.run_bass_kernel_spmd(nc, in_maps, core_ids=list(range(8)))
    return _assemble(res.results)


def run_traced(inputs):
    """Timing/profiling helper for test.py (not used by the grader)."""
    nc = _get_nc()
    in_maps = _make_in_maps(
        np.asarray(inputs["x"], np.float32),
        np.asarray(inputs["cos"], np.float32),
        np.asarray(inputs["sin"], np.float32),
        np.asarray(inputs["wq"], np.float32),
        np.asarray(inputs["wk"], np.float32),
        np.asarray(inputs["wv"], np.float32),
        np.asarray(inputs["wo"], np.float32),
    )
    res = bass_utils.run_bass_kernel_spmd(
        nc, in_maps, core_ids=list(range(8)), trace=True
    )
    return res
